# revision 59
# baseline (speedup 1.0000x reference)
"""Trainium2 Bass kernel for causal multi-head attention with adaptive
temperature (entropy-polynomial) softmax.

Problem shape: x [2, 2048, 1024], 16 heads x 64 dims, causal.
  q/k/v = x @ W{q,k,v}.T ; sim = q k^T / 8 (causal) ;
  attn = softmax(beta * sim), beta = f(entropy(softmax(sim))) ;
  out = (attn v) @ Wo.T + bo

Sharding (8 cores): core c owns batch b = c // 4 and heads
4*(c%4) .. 4*(c%4)+3.  Each core computes its heads' q/k/v projections
(tensor-parallel over the head dim), full [n, n] score tiles for its
heads, and a partial output projection over its 256 channel dims.
Host sums the 4 partials per batch and adds bo.

v2 design notes (engine balance; all matmuls bf16):
  The kernel is jointly limited by the PE (matmul rows stream at
  0.42ns/row only in long wait-free stretches) and the ACT engine
  (exp is 1 elem/cycle/lane @1.2GHz, two full causal passes are
  compulsory).  Division of labor:
    PE:   projections, scores (B1 [i,j] + B2 [j,i]), attn@V, out proj,
          causal diag masks (identity-matmul + mask accumulation)
    ACT:  exp only (B1 exp w/ Z1 accum; B2 exp), plus one tiny Ln
    DVE:  psum evacuations, B1's D = sum(s*e^s) via stt accum, stats
          algebra, Z2 recip, Z2/beta broadcast multiplies
  (GpSimd/Pool and the DMA engines cannot touch PSUM on TRN2, so all
  PSUM evacuation is split between DVE and ACT; phase C's evac is the
  split one.  1/Z2 is computed by a reciprocal that reads the Z2 row
  straight out of the attn@V psum accumulator.)
  B1 ascends rb and is interleaved into phase A's projection chains so
  ACT starts exp'ing ~10us into the kernel.  B2 sweeps i in 512-wide
  quarters (avp psum = 1 bank/head), with the two heads of an m-pair
  packed side-by-side in one [128,1024] psum tile so one strided exp
  covers both.  Z2 normalization broadcasts 1/Z2 over each head's 64
  channels with a selector matmul (no DMA broadcast).  Phase C runs
  per-quarter, interleaved into B2, output DMA'd per 512 rows.
"""

import numpy as np
import ml_dtypes

import concourse.bass as bass
import concourse.tile as tile
from concourse import bacc, mybir
from concourse.bass_utils import run_bass_kernel_spmd
from concourse.masks import make_identity

F32 = mybir.dt.float32
F32R = mybir.dt.float32r
BF16 = mybir.dt.bfloat16
I32 = mybir.dt.int32
AFT = mybir.ActivationFunctionType
ALU = mybir.AluOpType

B, N, DIM = 2, 2048, 1024
H_TOT, HD = 16, 64
N_CORES = 8
NH = 4            # heads per core
CD = NH * HD      # 256 channel dims per core
NRB = N // 128    # 16 row blocks
NU = NRB * NH     # 64 (rb, head) units
POLY = [-0.037, 0.481, -2.3, 4.917, -1.791]
MASK_VAL = -1e30
KC = DIM // 128   # 8 contraction chunks


def build_kernel(num_devices=N_CORES):
    nc = bacc.Bacc("TRN2", target_bir_lowering=False, debug=False,
                   num_devices=num_devices)

    xT = nc.dram_tensor("xT", [DIM, N], BF16, kind="ExternalInput").ap()
    wqT = nc.dram_tensor("wqT", [DIM, CD], BF16, kind="ExternalInput").ap()
    wkT = nc.dram_tensor("wkT", [DIM, CD], BF16, kind="ExternalInput").ap()
    wvT = nc.dram_tensor("wvT", [DIM, CD], BF16, kind="ExternalInput").ap()
    woT = nc.dram_tensor("woT", [CD, DIM], BF16, kind="ExternalInput").ap()
    maskin = nc.dram_tensor("maskin", [128, 128], BF16, kind="ExternalInput").ap()
    maskT2in = nc.dram_tensor("maskT2in", [128, 256], BF16, kind="ExternalInput").ap()
    selin = nc.dram_tensor("selin", [64, 32 * 128], BF16, kind="ExternalInput").ap()
    selzin = nc.dram_tensor("selzin", [128, 256], BF16, kind="ExternalInput").ap()
    partial = nc.dram_tensor("partial", [N, DIM], BF16, kind="ExternalOutput").ap()

    with tile.TileContext(nc) as tc:
        # ---- persistent pools (allocated first = live whole kernel) ----
        with tc.tile_pool(name="const", bufs=1) as constp, \
             tc.tile_pool(name="qkv_sb", bufs=1) as qkvp, \
             tc.tile_pool(name="attn_out", bufs=1) as aop, \
             tc.tile_pool(name="wo_sb", bufs=1) as wop, \
             tc.tile_pool(name="statsall", bufs=1) as sap:

            identB = constp.tile([128, 128], BF16)
            make_identity(nc, identB[:])
            idenF = constp.tile([128, 128], F32)
            make_identity(nc, idenF[:])
            selW = constp.tile([64, 32 * 128], BF16)
            nc.sync.dma_start(selW[0:64, :], selin[:])
            selZ = constp.tile([128, 256], BF16)
            nc.sync.dma_start(selZ[:], selzin[:])
            mask = constp.tile([128, 128], BF16)
            nc.sync.dma_start(mask[:], maskin[:])
            maskT2 = constp.tile([128, 256], BF16)
            nc.sync.dma_start(maskT2[:], maskT2in[:])
            ones64 = constp.tile([128, NU], F32)
            nc.vector.memset(ones64[:], 1.0)
            # ones columns of v_ext (overwritten except col 64 per head)

            # persistent activations
            qT = [qkvp.tile([128, N], BF16, tag=f"qT{m}", name=f"qT{m}") for m in range(2)]
            kT = [qkvp.tile([128, N], BF16, tag=f"kT{m}", name=f"kT{m}") for m in range(2)]
            qpT = [qkvp.tile([128, N], BF16, tag=f"qpT{m}", name=f"qpT{m}") for m in range(2)]
            # v_ext: per j-block, [128, 4*(64+1)]: per head 64 v-cols + ones
            v_bf = [qkvp.tile([128, NH * (HD + 1)], BF16, tag=f"v{j}", name=f"v{j}")
                    for j in range(NRB)]
            for j in range(NRB):
                nc.gpsimd.memset(v_bf[j][:], 1.0)
            attTu = [aop.tile([128, N], BF16, tag=f"attTu{m}", name=f"attTu{m}")
                     for m in range(2)]
            woS = [wop.tile([128, DIM], BF16, tag=f"wo{m}", name=f"wo{m}") for m in range(2)]

            Z1a = sap.tile([128, NU], F32)
            D1a = sap.tile([128, NU], F32)
            beta_all = sap.tile([128, NU], F32)
            betaT = sap.tile([128, 128], BF16)
            z2catF = sap.tile([128, N], F32)
            z2invF = sap.tile([128, N], F32)
            z2invB = sap.tile([128, N], BF16)
            nc.vector.memset(z2catF[:], 1.0)
            nc.vector.memset(z2invB[:], 1.0)

            # ---- phase A (QKV projections) with B1 rb0-7 interleaved ----
            # B1 (entropy stats) per unit (rb, h): scores [i=128, j<=W] on
            # PE, one exp on ACT (accum -> Z1), one stt on Pool (accum -> D).
            with tc.tile_pool(name="t1p", bufs=1) as t1p, \
                 tc.tile_pool(name="s2p", bufs=1) as s2p:

                def emit_b1_unit(rb, h, scpool, tw):
                    W = 128 * (rb + 1)
                    u = NH * rb + h
                    m, base = h // 2, 64 * (h % 2)
                    q_l = qT[m][base:base + 64, 128 * rb:128 * (rb + 1)]
                    sc = scpool.tile([128, tw], F32, tag="sc")
                    for so in range(0, W, 512):
                        sw = min(512, W - so)
                        has_diag = (so + sw == W)
                        sw0 = sw - 128 if has_diag else sw
                        if sw0 > 0:
                            nc.tensor.matmul(
                                sc[:, so:so + sw0], q_l,
                                kT[m][base:base + 64, so:so + sw0],
                                start=True, stop=True)
                        if has_diag:
                            dg = slice(W - 128, W)
                            nc.tensor.matmul(
                                sc[:, dg], q_l,
                                kT[m][base:base + 64, dg],
                                start=True, stop=False)
                            nc.tensor.matmul(
                                sc[:, dg], identB[:], mask[:],
                                start=False, stop=True)
                    t1 = t1p.tile([128, 2048], BF16, tag="t1", bufs=3)
                    nc.scalar.activation(
                        t1[:, :W], sc[:, :W], AFT.Exp,
                        bias=0.0, scale=1.0,
                        accum_out=Z1a[:, u:u + 1])
                    s2 = s2p.tile([128, 2048], BF16, tag="s2", bufs=2)
                    nc.vector.scalar_tensor_tensor(
                        out=s2[:, :W], in0=sc[:, :W], scalar=1.0,
                        in1=t1[:, :W], op0=ALU.mult, op1=ALU.mult,
                        accum_out=D1a[:, u:u + 1])

                with tc.tile_pool(name="xw_sb", bufs=1) as xwp, \
                     tc.tile_pool(name="qk_ps", bufs=2, space="PSUM") as qkps, \
                     tc.tile_pool(name="b1s_ps", bufs=2, space="PSUM") as b1sp:
                    xTs = [xwp.tile([128, N], BF16, tag=f"xT{k}", name=f"xTs{k}")
                           for k in range(KC)]
                    wq_s = [xwp.tile([128, CD], BF16, tag=f"wq{k}", name=f"wq{k}")
                            for k in range(KC)]
                    wk_s = [xwp.tile([128, CD], BF16, tag=f"wk{k}", name=f"wk{k}")
                            for k in range(KC)]
                    wv_s = [xwp.tile([128, CD], BF16, tag=f"wv{k}", name=f"wv{k}")
                            for k in range(KC)]
                    # front-load k=0 so the first accumulation chain starts
                    # early; spread issue load over several DMA queues
                    nc.scalar.dma_start(xTs[0][:], xT[0:128, :])
                    nc.sync.dma_start(wk_s[0][:], wkT[0:128, :])
                    nc.gpsimd.dma_start(wq_s[0][:], wqT[0:128, :])
                    for k in range(KC):
                        sl = slice(128 * k, 128 * (k + 1))
                        if k > 0:
                            nc.sync.dma_start(wk_s[k][:], wkT[sl, :])
                            nc.scalar.dma_start(wq_s[k][:], wqT[sl, :])
                        nc.sync.dma_start(wv_s[k][:], wvT[sl, :])
                        if k > 0:
                            nc.scalar.dma_start(xTs[k][:], xT[sl, :])
                    for m in range(2):
                        nc.sync.dma_start(woS[m][:], woT[128 * m:128 * (m + 1), :])

                    def emit_proj_block(wt, dest, m, nn):
                        pq = qkps.tile([128, 512], F32, tag="pq", bufs=4)
                        for k in range(KC):
                            nc.tensor.matmul(
                                pq[:], wt[k][:, 128 * m:128 * (m + 1)],
                                xTs[k][:, 512 * nn:512 * (nn + 1)],
                                start=(k == 0), stop=(k == KC - 1))
                        nc.vector.tensor_copy(
                            dest[m][:, 512 * nn:512 * (nn + 1)], pq[:])

                    for nn in range(4):
                        for m in range(2):
                            emit_proj_block(wk_s, kT, m, nn)
                        for m in range(2):
                            emit_proj_block(wq_s, qT, m, nn)
                        # B1 small units (W <= 1024) as soon as inputs exist
                        if nn < 2:
                            for rb in range(4 * nn, 4 * nn + 4):
                                for h in range(NH):
                                    emit_b1_unit(rb, h, b1sp, 1024)

                    # v: [j, d] = sum_c xT[c,j] wvT[c,d]; pack into v_ext
                    # with a ones column per head (col 65h+64)
                    for jt in range(NRB):
                        pvt = qkps.tile([128, 512], F32, tag="pq", bufs=4)
                        pv = pvt[:, 0:CD]
                        for k in range(KC):
                            nc.tensor.matmul(
                                pv, xTs[k][:, 128 * jt:128 * (jt + 1)],
                                wv_s[k][:],
                                start=(k == 0), stop=(k == KC - 1))
                        src3 = pv.rearrange("p (h c) -> p h c", h=NH)
                        dst3 = v_bf[jt].rearrange("p (h c) -> p h c",
                                                  h=NH)[:, :, 0:HD]
                        nc.vector.tensor_copy(dst3, src3)

                # ---- B1 large units (W > 1024) ----
                with tc.tile_pool(name="b1l_ps", bufs=2, space="PSUM") as b1lp:
                    for rb in range(8, NRB):
                        for h in range(NH):
                            emit_b1_unit(rb, h, b1lp, 2048)

            # ---- batched stats: H = ln Z1 - D/Z1 ; beta poly ----
            with tc.tile_pool(name="stats", bufs=4) as stp, \
                 tc.tile_pool(name="tr_ps", bufs=4, space="PSUM") as trps:
                rz = stp.tile([128, NU], F32, tag="rz")
                nc.vector.reciprocal(rz[:], Z1a[:])
                dn = stp.tile([128, NU], F32, tag="dn")
                nc.vector.tensor_mul(dn[:], D1a[:], rz[:])
                lnz = stp.tile([128, NU], F32, tag="lnz")
                nc.scalar.activation(lnz[:], Z1a[:], AFT.Ln, bias=0.0, scale=1.0)
                Hent = stp.tile([128, NU], F32, tag="Hent")
                nc.vector.tensor_sub(Hent[:], lnz[:], dn[:])
                p0 = stp.tile([128, NU], F32, tag="p0")
                nc.vector.tensor_scalar(out=p0[:], in0=Hent[:], scalar1=POLY[0],
                                        scalar2=POLY[1], op0=ALU.mult, op1=ALU.add)
                p1 = stp.tile([128, NU], F32, tag="p1")
                for c in POLY[2:]:
                    nc.vector.tensor_mul(p1[:], p0[:], Hent[:])
                    nc.vector.tensor_scalar_add(p0[:], p1[:], c)
                nc.vector.tensor_scalar_max(p1[:], p0[:], 1.0)
                mk = stp.tile([128, NU], I32, tag="mk")
                nc.vector.tensor_scalar(out=mk[:], in0=Hent[:], scalar1=0.5,
                                        scalar2=None, op0=ALU.is_gt)
                nc.vector.tensor_copy(beta_all[:], ones64[:])
                nc.vector.copy_predicated(beta_all[:], mk[:], p1[:])

                # betaT[u, i] via PE transpose; broadcast along each head's
                # 64 channel partitions via selector matmuls; q' = q * beta
                btp = trps.tile([128, 128], F32, tag="btp")
                nc.tensor.transpose(btp[0:NU, :], beta_all[:], idenF[:])
                nc.vector.tensor_copy(betaT[0:NU, :], btp[0:NU, :])
                for m in range(2):
                    bc_ps = trps.tile([128, 2048], F32, tag="bc_ps", bufs=1)
                    for rb in range(NRB):
                        csl = slice(128 * rb, 128 * (rb + 1))
                        ssl = slice(128 * (2 * rb + m), 128 * (2 * rb + m + 1))
                        nc.tensor.matmul(
                            bc_ps[:, csl], selW[0:64, ssl], betaT[0:64, :],
                            start=True, stop=True)
                    nc.vector.tensor_mul(qpT[m][:], qT[m][:], bc_ps[:])

            # ---- phase B2 (rescore + attn@V) + phase C, by i-quarters ----
            # per (q, m): stationary k j-blocks stream q' over i in
            # [512q, 512q+512); the two heads of the m-pair go side by side
            # in one [128, 1024] psum tile (one strided exp serves both);
            # attn^T @ v_ext accumulates [65, 512] per head over j-blocks
            # (ones column -> Z2); then 1/Z2 via one reciprocal + selector-
            # matmul broadcast; phase C projects the quarter and DMAs out.
            with tc.tile_pool(name="exp_sb", bufs=1) as exp_sb, \
                 tc.tile_pool(name="ost_sb", bufs=1) as ostp, \
                 tc.tile_pool(name="b2_ps", bufs=2, space="PSUM") as b2ps, \
                 tc.tile_pool(name="av_ps", bufs=1, space="PSUM") as avps:
                # Norm + phase C of quarter q are DEFERRED: their items are
                # injected one-per-visit into quarter q+1's sweep so the C
                # matmuls fill PE bubbles while ACT keeps streaming exps.
                deferred = []

                def drain_one():
                    if deferred:
                        deferred.pop(0)()

                def emit_quarter(q):
                    qc = slice(512 * q, 512 * (q + 1))
                    njt = 4 * q + 4
                    # both m-pairs interleaved jt-by-jt: doubles the PE work
                    # per dependency window so the score matmuls stop waiting
                    avp = {}
                    for m in range(2):
                        for hh in range(2):
                            avp[(m, hh)] = avps.tile(
                                [128, 512], F32, tag=f"avp{m}{hh}",
                                name=f"avp{m}{hh}")
                            if q == 0:
                                nc.vector.memset(avp[(m, hh)][:], 0.0)

                    def emit_av(m, jt, loc0, ex):
                        for hh in range(2):
                            h = 2 * m + hh
                            vl = v_bf[jt][:, (HD + 1) * h:(HD + 1) * (h + 1)]
                            nc.tensor.matmul(
                                avp[(m, hh)][0:HD + 1, loc0:512], vl,
                                ex[:, 512 * hh + loc0:512 * hh + 512],
                                start=(q > 0 and jt == 0),
                                stop=(jt == njt - 1),
                                skip_group_check=True)

                    pend = []
                    for jt in range(njt):
                        loc0 = max(0, 128 * jt - 512 * q)
                        is_diag = (128 * jt >= 512 * q)
                        for m in range(2):
                            sc2 = b2ps.tile([128, 1024], F32, tag="sc2")
                            for tb, base in ((0, 0), (1, 64)):
                                kjt = kT[m][base:base + 64,
                                            128 * jt:128 * (jt + 1)]
                                o0 = 512 * tb + loc0
                                if is_diag:
                                    # diag 128-block: score + identity-matmul
                                    # mask accumulation, rest separately
                                    nc.tensor.matmul(
                                        sc2[:, o0:o0 + 128], kjt,
                                        qpT[m][base:base + 64,
                                               512 * q + loc0:
                                               512 * q + loc0 + 128],
                                        start=True, stop=False)
                                    nc.tensor.matmul(
                                        sc2[:, o0:o0 + 128], identB[:],
                                        maskT2[:, 0:128],
                                        start=False, stop=True)
                                    if loc0 + 128 < 512:
                                        nc.tensor.matmul(
                                            sc2[:, o0 + 128:512 * (tb + 1)],
                                            kjt,
                                            qpT[m][base:base + 64,
                                                   512 * q + loc0 + 128:
                                                   512 * (q + 1)],
                                            start=True, stop=True)
                                else:
                                    nc.tensor.matmul(
                                        sc2[:, o0:512 * (tb + 1)], kjt,
                                        qpT[m][base:base + 64,
                                               512 * q + loc0:512 * (q + 1)],
                                        start=True, stop=True)
                            ex = exp_sb.tile([128, 1024], BF16, tag="ex",
                                             bufs=8)
                            nc.scalar.activation(
                                ex.rearrange("p (t c) -> p t c",
                                             t=2)[:, :, loc0:512],
                                sc2.rearrange("p (t c) -> p t c",
                                              t=2)[:, :, loc0:512],
                                AFT.Exp, bias=0.0, scale=1.0)
                            pend.append((m, jt, loc0, ex))
                            drain_one()
                            if len(pend) > 4:
                                emit_av(*pend.pop(0))
                    for p_ in pend:
                        emit_av(*p_)
                    # Z2 ones-row -> sbuf; attn@v -> attTu
                    for m in range(2):
                        for hh in range(2):
                            h = 2 * m + hh
                            nc.vector.tensor_copy(
                                z2catF[32 * h:32 * h + 1, qc],
                                avp[(m, hh)][HD:HD + 1, :])
                            nc.vector.tensor_copy(
                                attTu[m][64 * hh:64 * (hh + 1), qc],
                                avp[(m, hh)][0:64, :])

                    # build this quarter's deferred norm + phase C items
                    def norm_item(qc=qc):
                        # batched 1/Z2 (approx, ~18 bits) + bf16 convert
                        nc.vector.reciprocal_approx_fast(
                            out=z2invF[:, qc], in_=z2catF[:, qc])
                        with nc.allow_low_precision(reason="1/Z2 bf16"):
                            nc.vector.tensor_copy(z2invB[:, qc], z2invF[:, qc])
                    deferred.append(norm_item)

                    def zb_item(m, qc=qc):
                        zb = b2ps.tile([128, 1024], F32, tag="sc2")
                        nc.tensor.matmul(zb[:, 0:512],
                                         selZ[:, 128 * m:128 * (m + 1)],
                                         z2invB[:, qc], start=True, stop=True)
                        nc.vector.tensor_mul(attTu[m][:, qc], attTu[m][:, qc],
                                             zb[:, 0:512])
                    for m in range(2):
                        deferred.append(lambda m=m: zb_item(m))

                    ost = ostp.tile([128, 4 * DIM], BF16, tag="ost", bufs=2)

                    def c_item(rl, q=q, ost=ost):
                        rb = 4 * q + rl
                        pp = b2ps.tile([128, 1024], F32, tag="sc2")
                        for nn2 in range(2):
                            for m in range(2):
                                nc.tensor.matmul(
                                    pp[:, 512 * nn2:512 * (nn2 + 1)],
                                    attTu[m][:, 128 * rb:128 * (rb + 1)],
                                    woS[m][:, 512 * nn2:512 * (nn2 + 1)],
                                    start=(m == 0), stop=(m == 1))
                        dst = ost[:, DIM * rl:DIM * (rl + 1)]
                        if rl % 2 == 0:
                            nc.vector.tensor_copy(dst, pp[:])
                        else:
                            nc.scalar.copy(dst, pp[:])
                    for rl in range(4):
                        deferred.append(lambda rl=rl: c_item(rl))

                    def dma_item(qc=qc, ost=ost):
                        src = ost.rearrange("p (r f) -> p r f", r=4)
                        dst = partial[qc, :].rearrange("(r p) f -> p r f", r=4)
                        nc.sync.dma_start(dst, src)
                    deferred.append(dma_item)

                for q in range(4):
                    emit_quarter(q)
                while deferred:
                    deferred.pop(0)()

    nc.compile()
    return nc


_NC_CACHE = None
_LAST_IN_MAPS = None


def _make_in_maps(x, Wq, Wk, Wv, Wo):
    bf16 = ml_dtypes.bfloat16
    mask_h = np.where(np.arange(128)[None, :] > np.arange(128)[:, None],
                      np.float32(MASK_VAL), np.float32(0.0)).astype(bf16)
    maskT_h = np.ascontiguousarray(mask_h.T)
    maskT2_h = np.ascontiguousarray(np.concatenate([maskT_h, maskT_h], axis=1))
    woT_full = np.ascontiguousarray(Wo.T)  # [c, o]

    # selector for the beta channel-broadcast matmuls:
    # selW[u, 128*(2rb+m)+c] = 1 iff u == 4rb+2m+c//64
    sel_h = np.zeros((64, 32 * 128), dtype=np.float32)
    for rb in range(NRB):
        for m in range(2):
            b0 = 128 * (2 * rb + m)
            sel_h[4 * rb + 2 * m, b0:b0 + 64] = 1.0
            sel_h[4 * rb + 2 * m + 1, b0 + 64:b0 + 128] = 1.0
    sel_h = sel_h.astype(bf16)

    # selector for the 1/Z2 channel-broadcast matmuls:
    # selz[p, 128m + c] = 1 iff p == 32*(2m + c//64)
    selz_h = np.zeros((128, 256), dtype=np.float32)
    for m in range(2):
        for hh in range(2):
            selz_h[32 * (2 * m + hh),
                   128 * m + 64 * hh:128 * m + 64 * (hh + 1)] = 1.0
    selz_h = selz_h.astype(bf16)

    in_maps = []
    for c in range(N_CORES):
        b = c // 4
        s0 = CD * (c % 4)
        sl = slice(s0, s0 + CD)
        in_maps.append({
            "xT": np.ascontiguousarray(x[b].T).astype(bf16),
            "wqT": np.ascontiguousarray((Wq[sl, :] * 0.125).T).astype(bf16),
            "wkT": np.ascontiguousarray(Wk[sl, :].T).astype(bf16),
            "wvT": np.ascontiguousarray(Wv[sl, :].T).astype(bf16),
            "woT": np.ascontiguousarray(woT_full[sl, :]).astype(bf16),
            "maskin": mask_h,
            "maskT2in": maskT2_h,
            "selin": sel_h,
            "selzin": selz_h,
        })
    return in_maps


def kernel(x, Wq, Wk, Wv, Wo, bo):
    global _NC_CACHE, _LAST_IN_MAPS
    x = np.asarray(x, dtype=np.float32)
    Wq = np.asarray(Wq, dtype=np.float32)
    Wk = np.asarray(Wk, dtype=np.float32)
    Wv = np.asarray(Wv, dtype=np.float32)
    Wo = np.asarray(Wo, dtype=np.float32)
    bo = np.asarray(bo, dtype=np.float32)

    if _NC_CACHE is None:
        _NC_CACHE = build_kernel()
    nc = _NC_CACHE

    in_maps = _make_in_maps(x, Wq, Wk, Wv, Wo)
    _LAST_IN_MAPS = in_maps
    res = run_bass_kernel_spmd(nc, in_maps, core_ids=list(range(N_CORES)))

    out = np.zeros((B, N, DIM), dtype=np.float32)
    for c in range(N_CORES):
        out[c // 4] += res.results[c]["partial"].astype(np.float32)
    out += bo[None, None, :]
    return out


# revision 62
# speedup vs baseline: 1.0729x; 1.0729x over previous
"""Trainium2 Bass kernel for causal multi-head attention with adaptive
temperature (entropy-polynomial) softmax.

Problem shape: x [2, 2048, 1024], 16 heads x 64 dims, causal.
  q/k/v = x @ W{q,k,v}.T ; sim = q k^T / 8 (causal) ;
  attn = softmax(beta * sim), beta = f(entropy(softmax(sim))) ;
  out = (attn v) @ Wo.T + bo

Sharding (8 cores): core c owns batch b = c // 4 and heads
4*(c%4) .. 4*(c%4)+3.  Each core computes its heads' q/k/v projections
(tensor-parallel over the head dim), full [n, n] score tiles for its
heads, and a partial output projection over its 256 channel dims.
Host sums the 4 partials per batch and adds bo.

v2 design notes (engine balance; all matmuls bf16):
  The kernel is jointly limited by the PE (matmul rows stream at
  0.42ns/row only in long wait-free stretches) and the ACT engine
  (exp is 1 elem/cycle/lane @1.2GHz, two full causal passes are
  compulsory).  Division of labor:
    PE:   projections, scores (B1 [i,j] + B2 [j,i]), attn@V, out proj,
          causal diag masks (identity-matmul + mask accumulation)
    ACT:  exp only (B1 exp w/ Z1 accum; B2 exp), plus one tiny Ln
    DVE:  psum evacuations, B1's D = sum(s*e^s) via stt accum, stats
          algebra, Z2 recip, Z2/beta broadcast multiplies
  (GpSimd/Pool and the DMA engines cannot touch PSUM on TRN2, so all
  PSUM evacuation is split between DVE and ACT; phase C's evac is the
  split one.  1/Z2 is computed by a reciprocal that reads the Z2 row
  straight out of the attn@V psum accumulator.)
  B1 ascends rb and is interleaved into phase A's projection chains so
  ACT starts exp'ing ~10us into the kernel.  B2 sweeps i in 512-wide
  quarters (avp psum = 1 bank/head), with the two heads of an m-pair
  packed side-by-side in one [128,1024] psum tile so one strided exp
  covers both.  Z2 normalization broadcasts 1/Z2 over each head's 64
  channels with a selector matmul (no DMA broadcast).  Phase C runs
  per-quarter, interleaved into B2, output DMA'd per 512 rows.
"""

import numpy as np
import ml_dtypes

import concourse.bass as bass
import concourse.tile as tile
from concourse import bacc, mybir
from concourse.bass_utils import run_bass_kernel_spmd
from concourse.masks import make_identity

F32 = mybir.dt.float32
F32R = mybir.dt.float32r
BF16 = mybir.dt.bfloat16
I32 = mybir.dt.int32
AFT = mybir.ActivationFunctionType
ALU = mybir.AluOpType

B, N, DIM = 2, 2048, 1024
H_TOT, HD = 16, 64
N_CORES = 8
NH = 4            # heads per core
CD = NH * HD      # 256 channel dims per core
NRB = N // 128    # 16 row blocks
NU = NRB * NH     # 64 (rb, head) units
POLY = [-0.037, 0.481, -2.3, 4.917, -1.791]
MASK_VAL = -1e30
KC = DIM // 128   # 8 contraction chunks


def build_kernel(num_devices=N_CORES):
    nc = bacc.Bacc("TRN2", target_bir_lowering=False, debug=False,
                   num_devices=num_devices)

    xT = nc.dram_tensor("xT", [DIM, N], BF16, kind="ExternalInput").ap()
    wqT = nc.dram_tensor("wqT", [DIM, CD], BF16, kind="ExternalInput").ap()
    wkT = nc.dram_tensor("wkT", [DIM, CD], BF16, kind="ExternalInput").ap()
    wvT = nc.dram_tensor("wvT", [DIM, CD], BF16, kind="ExternalInput").ap()
    woT = nc.dram_tensor("woT", [CD, DIM], BF16, kind="ExternalInput").ap()
    maskin = nc.dram_tensor("maskin", [128, 128], BF16, kind="ExternalInput").ap()
    maskT2in = nc.dram_tensor("maskT2in", [128, 256], BF16, kind="ExternalInput").ap()
    selin = nc.dram_tensor("selin", [64, 32 * 128], BF16, kind="ExternalInput").ap()
    selzin = nc.dram_tensor("selzin", [128, 256], BF16, kind="ExternalInput").ap()
    partial = nc.dram_tensor("partial", [N, DIM], BF16, kind="ExternalOutput").ap()

    with tile.TileContext(nc) as tc:
        # ---- persistent pools (allocated first = live whole kernel) ----
        with tc.tile_pool(name="const", bufs=1) as constp, \
             tc.tile_pool(name="qkv_sb", bufs=1) as qkvp, \
             tc.tile_pool(name="attn_out", bufs=1) as aop, \
             tc.tile_pool(name="wo_sb", bufs=1) as wop, \
             tc.tile_pool(name="statsall", bufs=1) as sap:

            identB = constp.tile([128, 128], BF16)
            make_identity(nc, identB[:])
            idenF = constp.tile([128, 128], F32)
            make_identity(nc, idenF[:])
            selW = constp.tile([64, 32 * 128], BF16)
            nc.sync.dma_start(selW[0:64, :], selin[:])
            selZ = constp.tile([128, 256], BF16)
            nc.sync.dma_start(selZ[:], selzin[:])
            mask = constp.tile([128, 128], BF16)
            nc.sync.dma_start(mask[:], maskin[:])
            maskT2 = constp.tile([128, 256], BF16)
            nc.sync.dma_start(maskT2[:], maskT2in[:])
            ones64 = constp.tile([128, NU], F32)
            nc.vector.memset(ones64[:], 1.0)
            # ones columns of v_ext (overwritten except col 64 per head)

            # persistent activations
            qT = [qkvp.tile([128, N], BF16, tag=f"qT{m}", name=f"qT{m}") for m in range(2)]
            kT = [qkvp.tile([128, N], BF16, tag=f"kT{m}", name=f"kT{m}") for m in range(2)]
            qpT = [qkvp.tile([128, N], BF16, tag=f"qpT{m}", name=f"qpT{m}") for m in range(2)]
            # v_ext: per j-block, [128, 4*(64+1)]: per head 64 v-cols + ones
            v_bf = [qkvp.tile([128, NH * (HD + 1)], BF16, tag=f"v{j}", name=f"v{j}")
                    for j in range(NRB)]
            for j in range(NRB):
                nc.gpsimd.memset(v_bf[j][:], 1.0)
            attTu = [aop.tile([128, N], BF16, tag=f"attTu{m}", name=f"attTu{m}")
                     for m in range(2)]
            woS = [wop.tile([128, DIM], BF16, tag=f"wo{m}", name=f"wo{m}") for m in range(2)]

            Z1a = sap.tile([128, NU], F32)
            D1a = sap.tile([128, NU], F32)
            beta_all = sap.tile([128, NU], F32)
            betaT = sap.tile([128, 128], BF16)
            z2catF = sap.tile([128, N], F32)
            z2invF = sap.tile([128, N], F32)
            z2invB = sap.tile([128, N], BF16)
            nc.vector.memset(z2catF[:], 1.0)
            nc.vector.memset(z2invB[:], 1.0)

            # ---- phase A (QKV projections) with B1 rb0-7 interleaved ----
            # B1 (entropy stats) per unit (rb, h): scores [i=128, j<=W] on
            # PE, one exp on ACT (accum -> Z1), one stt on Pool (accum -> D).
            with tc.tile_pool(name="t1p", bufs=1) as t1p, \
                 tc.tile_pool(name="s2p", bufs=1) as s2p:

                def emit_b1_unit(rb, h, scpool, tw):
                    W = 128 * (rb + 1)
                    u = NH * rb + h
                    m, base = h // 2, 64 * (h % 2)
                    q_l = qT[m][base:base + 64, 128 * rb:128 * (rb + 1)]
                    sc = scpool.tile([128, tw], F32, tag="sc")
                    for so in range(0, W, 512):
                        sw = min(512, W - so)
                        has_diag = (so + sw == W)
                        sw0 = sw - 128 if has_diag else sw
                        if sw0 > 0:
                            nc.tensor.matmul(
                                sc[:, so:so + sw0], q_l,
                                kT[m][base:base + 64, so:so + sw0],
                                start=True, stop=True)
                        if has_diag:
                            dg = slice(W - 128, W)
                            nc.tensor.matmul(
                                sc[:, dg], q_l,
                                kT[m][base:base + 64, dg],
                                start=True, stop=False)
                            nc.tensor.matmul(
                                sc[:, dg], identB[:], mask[:],
                                start=False, stop=True)
                    t1 = t1p.tile([128, 2048], BF16, tag="t1", bufs=3)
                    nc.scalar.activation(
                        t1[:, :W], sc[:, :W], AFT.Exp,
                        bias=0.0, scale=1.0,
                        accum_out=Z1a[:, u:u + 1])
                    s2 = s2p.tile([128, 2048], BF16, tag="s2", bufs=2)
                    nc.vector.scalar_tensor_tensor(
                        out=s2[:, :W], in0=sc[:, :W], scalar=1.0,
                        in1=t1[:, :W], op0=ALU.mult, op1=ALU.mult,
                        accum_out=D1a[:, u:u + 1])

                with tc.tile_pool(name="xw_sb", bufs=1) as xwp, \
                     tc.tile_pool(name="qk_ps", bufs=2, space="PSUM") as qkps, \
                     tc.tile_pool(name="b1s_ps", bufs=2, space="PSUM") as b1sp:
                    xTs = [xwp.tile([128, N], BF16, tag=f"xT{k}", name=f"xTs{k}")
                           for k in range(KC)]
                    wq_s = [xwp.tile([128, CD], BF16, tag=f"wq{k}", name=f"wq{k}")
                            for k in range(KC)]
                    wk_s = [xwp.tile([128, CD], BF16, tag=f"wk{k}", name=f"wk{k}")
                            for k in range(KC)]
                    wv_s = [xwp.tile([128, CD], BF16, tag=f"wv{k}", name=f"wv{k}")
                            for k in range(KC)]
                    # front-load k=0 so the first accumulation chain starts
                    # early; spread issue load over several DMA queues
                    nc.scalar.dma_start(xTs[0][:], xT[0:128, :])
                    nc.sync.dma_start(wk_s[0][:], wkT[0:128, :])
                    nc.gpsimd.dma_start(wq_s[0][:], wqT[0:128, :])
                    for k in range(KC):
                        sl = slice(128 * k, 128 * (k + 1))
                        if k > 0:
                            nc.sync.dma_start(wk_s[k][:], wkT[sl, :])
                            nc.scalar.dma_start(wq_s[k][:], wqT[sl, :])
                        nc.sync.dma_start(wv_s[k][:], wvT[sl, :])
                        if k > 0:
                            nc.scalar.dma_start(xTs[k][:], xT[sl, :])
                    for m in range(2):
                        nc.sync.dma_start(woS[m][:], woT[128 * m:128 * (m + 1), :])

                    def emit_proj_block(wt, dest, m, nn):
                        pq = qkps.tile([128, 512], F32, tag="pq", bufs=4)
                        for k in range(KC):
                            nc.tensor.matmul(
                                pq[:], wt[k][:, 128 * m:128 * (m + 1)],
                                xTs[k][:, 512 * nn:512 * (nn + 1)],
                                start=(k == 0), stop=(k == KC - 1))
                        nc.vector.tensor_copy(
                            dest[m][:, 512 * nn:512 * (nn + 1)], pq[:])

                    for nn in range(4):
                        for m in range(2):
                            emit_proj_block(wk_s, kT, m, nn)
                        for m in range(2):
                            emit_proj_block(wq_s, qT, m, nn)
                        # B1 small units (W <= 1024) as soon as inputs exist
                        if nn < 2:
                            for rb in range(4 * nn, 4 * nn + 4):
                                for h in range(NH):
                                    emit_b1_unit(rb, h, b1sp, 1024)

                    # v: [j, d] = sum_c xT[c,j] wvT[c,d]; pack into v_ext
                    # with a ones column per head (col 65h+64)
                    for jt in range(NRB):
                        pvt = qkps.tile([128, 512], F32, tag="pq", bufs=4)
                        pv = pvt[:, 0:CD]
                        for k in range(KC):
                            nc.tensor.matmul(
                                pv, xTs[k][:, 128 * jt:128 * (jt + 1)],
                                wv_s[k][:],
                                start=(k == 0), stop=(k == KC - 1))
                        src3 = pv.rearrange("p (h c) -> p h c", h=NH)
                        dst3 = v_bf[jt].rearrange("p (h c) -> p h c",
                                                  h=NH)[:, :, 0:HD]
                        nc.vector.tensor_copy(dst3, src3)

                # ---- B1 large units (W > 1024) ----
                with tc.tile_pool(name="b1l_ps", bufs=2, space="PSUM") as b1lp:
                    for rb in range(8, NRB):
                        for h in range(NH):
                            emit_b1_unit(rb, h, b1lp, 2048)

            # ---- batched stats: H = ln Z1 - D/Z1 ; beta poly ----
            with tc.tile_pool(name="stats", bufs=4) as stp, \
                 tc.tile_pool(name="tr_ps", bufs=4, space="PSUM") as trps:
                rz = stp.tile([128, NU], F32, tag="rz")
                nc.vector.reciprocal(rz[:], Z1a[:])
                dn = stp.tile([128, NU], F32, tag="dn")
                nc.vector.tensor_mul(dn[:], D1a[:], rz[:])
                lnz = stp.tile([128, NU], F32, tag="lnz")
                nc.scalar.activation(lnz[:], Z1a[:], AFT.Ln, bias=0.0, scale=1.0)
                Hent = stp.tile([128, NU], F32, tag="Hent")
                nc.vector.tensor_sub(Hent[:], lnz[:], dn[:])
                p0 = stp.tile([128, NU], F32, tag="p0")
                nc.vector.tensor_scalar(out=p0[:], in0=Hent[:], scalar1=POLY[0],
                                        scalar2=POLY[1], op0=ALU.mult, op1=ALU.add)
                p1 = stp.tile([128, NU], F32, tag="p1")
                for c in POLY[2:]:
                    nc.vector.tensor_mul(p1[:], p0[:], Hent[:])
                    nc.vector.tensor_scalar_add(p0[:], p1[:], c)
                nc.vector.tensor_scalar_max(p1[:], p0[:], 1.0)
                mk = stp.tile([128, NU], I32, tag="mk")
                nc.vector.tensor_scalar(out=mk[:], in0=Hent[:], scalar1=0.5,
                                        scalar2=None, op0=ALU.is_gt)
                nc.vector.tensor_copy(beta_all[:], ones64[:])
                nc.vector.copy_predicated(beta_all[:], mk[:], p1[:])

                # betaT[u, i] via PE transpose; broadcast along each head's
                # 64 channel partitions via selector matmuls; q' = q * beta
                btp = trps.tile([128, 128], F32, tag="btp")
                nc.tensor.transpose(btp[0:NU, :], beta_all[:], idenF[:])
                nc.vector.tensor_copy(betaT[0:NU, :], btp[0:NU, :])
                for m in range(2):
                    bc_ps = trps.tile([128, 2048], F32, tag="bc_ps", bufs=1)
                    for rb in range(NRB):
                        csl = slice(128 * rb, 128 * (rb + 1))
                        ssl = slice(128 * (2 * rb + m), 128 * (2 * rb + m + 1))
                        nc.tensor.matmul(
                            bc_ps[:, csl], selW[0:64, ssl], betaT[0:64, :],
                            start=True, stop=True)
                    nc.vector.tensor_mul(qpT[m][:], qT[m][:], bc_ps[:])

            # ---- phase B2 (rescore + attn@V) + phase C, by i-quarters ----
            # per (q, m): stationary k j-blocks stream q' over i in
            # [512q, 512q+512); the two heads of the m-pair go side by side
            # in one [128, 1024] psum tile (one strided exp serves both);
            # attn^T @ v_ext accumulates [65, 512] per head over j-blocks
            # (ones column -> Z2); then 1/Z2 via one reciprocal + selector-
            # matmul broadcast; phase C projects the quarter and DMAs out.
            with tc.tile_pool(name="exp_sb", bufs=1) as exp_sb, \
                 tc.tile_pool(name="ost_sb", bufs=1) as ostp, \
                 tc.tile_pool(name="b2_ps", bufs=2, space="PSUM") as b2ps, \
                 tc.tile_pool(name="av_ps", bufs=1, space="PSUM") as avps, \
                 tc.tile_pool(name="sm_ps", bufs=2, space="PSUM") as smps:
                # Norm + phase C of quarter q are DEFERRED: their items are
                # injected one-per-visit into quarter q+1's sweep so the C
                # matmuls fill PE bubbles while ACT keeps streaming exps.
                deferred = []

                def drain_one():
                    if deferred:
                        deferred.pop(0)()

                def emit_quarter(q):
                    qc = slice(512 * q, 512 * (q + 1))
                    njt = 4 * q + 4
                    for m in range(2):
                        avp = [avps.tile([128, 512], F32, tag=f"avp{hh}",
                                         name=f"avp{hh}") for hh in range(2)]
                        if q == 0:
                            for hh in range(2):
                                nc.vector.memset(avp[hh][:], 0.0)

                        def emit_av(jt, loc0, ex):
                            for hh in range(2):
                                h = 2 * m + hh
                                vl = v_bf[jt][:, (HD + 1) * h:(HD + 1) * (h + 1)]
                                nc.tensor.matmul(
                                    avp[hh][0:HD + 1, loc0:512], vl,
                                    ex[:, 512 * hh + loc0:512 * hh + 512],
                                    start=(q > 0 and jt == 0),
                                    stop=(jt == njt - 1),
                                    skip_group_check=True)

                        pend = []
                        for jt in range(njt):
                            loc0 = max(0, 128 * jt - 512 * q)
                            is_diag = (128 * jt >= 512 * q)
                            sc2 = b2ps.tile([128, 1024], F32, tag="sc2")
                            for tb, base in ((0, 0), (1, 64)):
                                kjt = kT[m][base:base + 64,
                                            128 * jt:128 * (jt + 1)]
                                o0 = 512 * tb + loc0
                                if is_diag:
                                    # diag 128-block: score + identity-matmul
                                    # mask accumulation, rest separately
                                    nc.tensor.matmul(
                                        sc2[:, o0:o0 + 128], kjt,
                                        qpT[m][base:base + 64,
                                               512 * q + loc0:
                                               512 * q + loc0 + 128],
                                        start=True, stop=False)
                                    nc.tensor.matmul(
                                        sc2[:, o0:o0 + 128], identB[:],
                                        maskT2[:, 0:128],
                                        start=False, stop=True)
                                    if loc0 + 128 < 512:
                                        nc.tensor.matmul(
                                            sc2[:, o0 + 128:512 * (tb + 1)],
                                            kjt,
                                            qpT[m][base:base + 64,
                                                   512 * q + loc0 + 128:
                                                   512 * (q + 1)],
                                            start=True, stop=True)
                                else:
                                    nc.tensor.matmul(
                                        sc2[:, o0:512 * (tb + 1)], kjt,
                                        qpT[m][base:base + 64,
                                               512 * q + loc0:512 * (q + 1)],
                                        start=True, stop=True)
                            ex = exp_sb.tile([128, 1024], BF16, tag="ex",
                                             bufs=6)
                            nc.scalar.activation(
                                ex.rearrange("p (t c) -> p t c",
                                             t=2)[:, :, loc0:512],
                                sc2.rearrange("p (t c) -> p t c",
                                              t=2)[:, :, loc0:512],
                                AFT.Exp, bias=0.0, scale=1.0)
                            pend.append((jt, loc0, ex))
                            drain_one()
                            if len(pend) > 2:
                                emit_av(*pend.pop(0))
                        for p_ in pend:
                            emit_av(*p_)
                        # Z2 ones-row -> sbuf; attn@v -> attTu
                        for hh in range(2):
                            h = 2 * m + hh
                            nc.vector.tensor_copy(
                                z2catF[32 * h:32 * h + 1, qc],
                                avp[hh][HD:HD + 1, :])
                            nc.vector.tensor_copy(
                                attTu[m][64 * hh:64 * (hh + 1), qc],
                                avp[hh][0:64, :])

                    # build this quarter's deferred norm + phase C items
                    def norm_item(qc=qc):
                        # batched 1/Z2 (approx, ~18 bits) + bf16 convert
                        nc.vector.reciprocal_approx_fast(
                            out=z2invF[:, qc], in_=z2catF[:, qc])
                        with nc.allow_low_precision(reason="1/Z2 bf16"):
                            nc.vector.tensor_copy(z2invB[:, qc], z2invF[:, qc])
                    deferred.append(norm_item)

                    def zb_item(m, qc=qc):
                        zb = smps.tile([128, 512], F32, tag="sm")
                        nc.tensor.matmul(zb[:], selZ[:, 128 * m:128 * (m + 1)],
                                         z2invB[:, qc], start=True, stop=True)
                        nc.vector.tensor_mul(attTu[m][:, qc], attTu[m][:, qc],
                                             zb[:])
                    for m in range(2):
                        deferred.append(lambda m=m: zb_item(m))

                    ost = ostp.tile([128, 4 * DIM], BF16, tag="ost", bufs=2)

                    def c_item(rl, nn2, q=q, ost=ost):
                        rb = 4 * q + rl
                        pp = smps.tile([128, 512], F32, tag="sm")
                        for m in range(2):
                            nc.tensor.matmul(
                                pp[:], attTu[m][:, 128 * rb:128 * (rb + 1)],
                                woS[m][:, 512 * nn2:512 * (nn2 + 1)],
                                start=(m == 0), stop=(m == 1))
                        dst = ost[:, DIM * rl + 512 * nn2:
                                  DIM * rl + 512 * (nn2 + 1)]
                        if nn2 == 0:
                            nc.vector.tensor_copy(dst, pp[:])
                        else:
                            nc.scalar.copy(dst, pp[:])
                    for rl in range(4):
                        for nn2 in range(2):
                            deferred.append(
                                lambda rl=rl, nn2=nn2: c_item(rl, nn2))

                    def dma_item(qc=qc, ost=ost):
                        src = ost.rearrange("p (r f) -> p r f", r=4)
                        dst = partial[qc, :].rearrange("(r p) f -> p r f", r=4)
                        nc.sync.dma_start(dst, src)
                    deferred.append(dma_item)

                for q in range(4):
                    emit_quarter(q)
                while deferred:
                    deferred.pop(0)()

    nc.compile()
    return nc


_NC_CACHE = None
_LAST_IN_MAPS = None


def _make_in_maps(x, Wq, Wk, Wv, Wo):
    bf16 = ml_dtypes.bfloat16
    mask_h = np.where(np.arange(128)[None, :] > np.arange(128)[:, None],
                      np.float32(MASK_VAL), np.float32(0.0)).astype(bf16)
    maskT_h = np.ascontiguousarray(mask_h.T)
    maskT2_h = np.ascontiguousarray(np.concatenate([maskT_h, maskT_h], axis=1))
    woT_full = np.ascontiguousarray(Wo.T)  # [c, o]

    # selector for the beta channel-broadcast matmuls:
    # selW[u, 128*(2rb+m)+c] = 1 iff u == 4rb+2m+c//64
    sel_h = np.zeros((64, 32 * 128), dtype=np.float32)
    for rb in range(NRB):
        for m in range(2):
            b0 = 128 * (2 * rb + m)
            sel_h[4 * rb + 2 * m, b0:b0 + 64] = 1.0
            sel_h[4 * rb + 2 * m + 1, b0 + 64:b0 + 128] = 1.0
    sel_h = sel_h.astype(bf16)

    # selector for the 1/Z2 channel-broadcast matmuls:
    # selz[p, 128m + c] = 1 iff p == 32*(2m + c//64)
    selz_h = np.zeros((128, 256), dtype=np.float32)
    for m in range(2):
        for hh in range(2):
            selz_h[32 * (2 * m + hh),
                   128 * m + 64 * hh:128 * m + 64 * (hh + 1)] = 1.0
    selz_h = selz_h.astype(bf16)

    in_maps = []
    for c in range(N_CORES):
        b = c // 4
        s0 = CD * (c % 4)
        sl = slice(s0, s0 + CD)
        in_maps.append({
            "xT": np.ascontiguousarray(x[b].T).astype(bf16),
            "wqT": np.ascontiguousarray((Wq[sl, :] * 0.125).T).astype(bf16),
            "wkT": np.ascontiguousarray(Wk[sl, :].T).astype(bf16),
            "wvT": np.ascontiguousarray(Wv[sl, :].T).astype(bf16),
            "woT": np.ascontiguousarray(woT_full[sl, :]).astype(bf16),
            "maskin": mask_h,
            "maskT2in": maskT2_h,
            "selin": sel_h,
            "selzin": selz_h,
        })
    return in_maps


def kernel(x, Wq, Wk, Wv, Wo, bo):
    global _NC_CACHE, _LAST_IN_MAPS
    x = np.asarray(x, dtype=np.float32)
    Wq = np.asarray(Wq, dtype=np.float32)
    Wk = np.asarray(Wk, dtype=np.float32)
    Wv = np.asarray(Wv, dtype=np.float32)
    Wo = np.asarray(Wo, dtype=np.float32)
    bo = np.asarray(bo, dtype=np.float32)

    if _NC_CACHE is None:
        _NC_CACHE = build_kernel()
    nc = _NC_CACHE

    in_maps = _make_in_maps(x, Wq, Wk, Wv, Wo)
    _LAST_IN_MAPS = in_maps
    res = run_bass_kernel_spmd(nc, in_maps, core_ids=list(range(N_CORES)))

    out = np.zeros((B, N, DIM), dtype=np.float32)
    for c in range(N_CORES):
        out[c // 4] += res.results[c]["partial"].astype(np.float32)
    out += bo[None, None, :]
    return out
